# revision 30
# baseline (speedup 1.0000x reference)
"""Trainium2 Bass kernel for multi-head cross-attention (dense_transformer).

Reference (per batch element b):
    qh = (q @ w_q)  -> heads [n, h, dk];  kh = (k @ w_k);  vh = (v @ w_v)
    att = softmax(qh @ kh^T * TEMP);  out = (att @ vh) merged @ w_o + q

Distribution: pure data-parallel over batch B=8 across the 8 NeuronCores
(one batch element per core, zero collectives).

Per-core algorithm (fp8e4m3 DoubleRow matmuls everywhere except S=QK^T):
  - weights are pre-scaled by 8 during the fp32->fp8 cast so their values
    sit in the e4m3 normal range; the extra 64x on S folds into the exp
    scale, the 512x on (U/r)@w_o folds into the final residual-add.
  - k/v stream in 512-row chunks: fp32 DMA -> SBUF, fp8 cast (gpsimd),
    then an SBUF->SBUF xbar transpose of the fp8 data viewed as 16-bit
    pairs.  A pair (db=2u, db=2u+1) lands in one 16-bit unit on partition
    u, which is exactly the [p, 2, m] layout DoubleRow wants (contraction
    index db = half*256 + 2u + j).  No DRAM bounce.
  - kh^T[dk, m] = w_k8^T @ k^T   (2 DR matmuls per 512-chunk, fp32 psum)
  - vh  [m, hdv] = v @ w_v8      (DR, lhsT = transposed v pairs)
  - per head: S^T[m, n] in 2-subtile psum groups [128, 2, 512]; one ACT
    exp per group (scale=TEMP/64, bias=-2) -> fp8 E^T pairs; then
    U^T[dv, n] += vh-pair.T @ E^T (DR) ; r[1, n] += ones.T @ E^T (DR).
    Normalize: rec = 8/r via reciprocal_approx_fast, broadcast across
    partitions with an f32r outer-product on the PE; UT8 = psU * rec.
  - out = (UT8 @ w_o8)/512 + q   (DR over head-pairs; scalar_tensor_tensor
    fuses the 1/512 and the residual add).
  - head 0 (plus head 1's kh projection and all of the v projection) is
    woven into the k/v marshal stream chunk-by-chunk; heads 1..7 run at
    full PE rate from SBUF-resident kT/vh.
  - per-engine FIFO discipline: every cast/evac is emitted on an engine in
    (approximate) execution order of its *data arrival* so no instruction
    with a long wait blocks later-ready work on the same queue.
"""

from contextlib import ExitStack

import numpy as np

import concourse.bass as bass
import concourse.tile as tile
from concourse import bacc, mybir

F32 = mybir.dt.float32
F32R = mybir.dt.float32r
BF16 = mybir.dt.bfloat16
FP8 = mybir.dt.float8e4
EXP = mybir.ActivationFunctionType.Exp
COPY = mybir.ActivationFunctionType.Copy
MULT = mybir.AluOpType.mult
ADD = mybir.AluOpType.add
DR = mybir.MatmulPerfMode.DoubleRow

B = 8
N = 512          # latent tokens (rows of q)
M = 4096         # byte tokens (rows of k/v)
DL = 1024        # d_latent
DB = 512         # d_byte
H = 8
DK = 128
DV = 128
TEMP = 0.08838834764831845
WS = 8.0         # weight pre-scale (folded back out downstream)

DEBUG_DUMP = None
CAST_ENG = lambda nc: nc.gpsimd
CSTAGE_BUFS = 2
C8_BUFS = 2
VT_BUFS = 2

P = 128
MC = 512         # m-chunk (marshal + compute granularity)
NCH = M // MC    # 8 chunks
MS = M // P      # 32 m-subtiles
NG = MS // 2     # 16 groups of 2 subtiles per head
LAGG = 2         # PV trails S by this many groups


def _dr_rhs(t_u16):
    """[p, a, P] bf16 pair-tensor slice -> [p, 2, a*P] fp8 DoubleRow rhs."""
    return t_u16.bitcast(FP8).rearrange("u a (m j) -> u j (a m)", j=2)


def _dr_lhs(t_u16):
    """[p, P] bf16 pair-tensor slice -> [p, 2, P] fp8 DoubleRow lhsT."""
    return t_u16.bitcast(FP8).rearrange("u (m j) -> u j m", j=2)


def build_kernel(nc, tc):
    aq = nc.dram_tensor("q", [N, DL], F32, kind="ExternalInput").ap()
    ak = nc.dram_tensor("k", [M, DB], F32, kind="ExternalInput").ap()
    av = nc.dram_tensor("v", [M, DB], F32, kind="ExternalInput").ap()
    awq = nc.dram_tensor("w_q", [DL, H * DK], F32, kind="ExternalInput").ap()
    awk = nc.dram_tensor("w_k", [DB, H * DK], F32, kind="ExternalInput").ap()
    awv = nc.dram_tensor("w_v", [DB, H * DV], F32, kind="ExternalInput").ap()
    awo = nc.dram_tensor("w_o", [H * DV, DL], F32, kind="ExternalInput").ap()
    aout = nc.dram_tensor("out", [N, DL], F32, kind="ExternalOutput").ap()

    with ExitStack() as ctx:
        persist = ctx.enter_context(tc.tile_pool(name="persist", bufs=1))
        khtp = ctx.enter_context(tc.tile_pool(name="khtp", bufs=2))
        cstage = ctx.enter_context(tc.tile_pool(name="cstage", bufs=CSTAGE_BUFS))
        c8p = ctx.enter_context(tc.tile_pool(name="c8p", bufs=C8_BUFS))
        vT8p = ctx.enter_context(tc.tile_pool(name="vT8p", bufs=VT_BUFS))
        wstage = ctx.enter_context(tc.tile_pool(name="wstage", bufs=3))
        etp = ctx.enter_context(tc.tile_pool(name="etp", bufs=4))
        recp = ctx.enter_context(tc.tile_pool(name="recp", bufs=1))
        otp = ctx.enter_context(tc.tile_pool(name="otp", bufs=2))
        psSp = ctx.enter_context(tc.tile_pool(name="psS", bufs=2, space="PSUM"))
        psUp = ctx.enter_context(tc.tile_pool(name="psU", bufs=1, space="PSUM"))
        psRp = ctx.enter_context(tc.tile_pool(name="psR", bufs=1, space="PSUM"))
        misc = ctx.enter_context(tc.tile_pool(name="misc", bufs=2, space="PSUM"))

        # persistent tensors ------------------------------------------------
        q8 = persist.tile([P, DL // 256, (N // P) * P], BF16)  # 4KB
        qT8u = persist.tile([P, DL // 256, N // P, P], BF16)  # q^T fp8 pairs 4KB
        kT8u = persist.tile([P, DB // 256, NCH, MC // P, P], BF16)  # k^T   16KB
        wq8 = persist.tile([P, DL // 256, 2, H * DK], FP8)    # 8KB
        wk8 = persist.tile([P, DB // 256, 2, H * DK], FP8)    # 4KB
        wv16 = persist.tile([P, DB // P, H * DV], BF16)       # 8KB
        wo8 = persist.tile([P, (H * DV) // 256, 2, DL], FP8)  # 8KB
        qhT = persist.tile([P, H, N], FP8)                    # 4KB
        vh = persist.tile([P, MS, H * DV], FP8)               # 32KB
        UT8 = persist.tile([P, H, N], FP8)                    # 4KB
        ones8 = persist.tile([P, 2, 16], FP8)   # lhsT slice [:, :, 0:1]: j-step 16
        onesr = persist.tile([1, P], BF16)
        biasT = persist.tile([P, 1], F32)
        nc.vector.memset(ones8, 1.0)
        nc.vector.memset(onesr, WS)        # folds UT8 = 8 * psU / r
        nc.vector.memset(biasT, -3.5)

        # ---- weight DMAs: scalar HWDGE queue, need-ordered ---------------
        def w_dma(src_ap, halves, width, tag, dma_eng, pat="(h u j) c -> u h j c"):
            src = src_ap.rearrange(pat, h=halves, u=P, j=2)
            tiles = []
            for h in range(halves):
                for j in range(2):
                    ws = wstage.tile([P, width], F32, tag="ws",
                                     name=f"ws_{tag}_{h}_{j}")
                    dma_eng.dma_start(out=ws, in_=src[:, h, j])
                    tiles.append((h, j, ws))
            return tiles

        wq_src = awq.rearrange("(h u j) c -> u h j c", h=DL // 256, u=P, j=2)
        wq_st = []
        for h in range(DL // 256):
            for j in range(2):
                ws = wstage.tile([P, H * DK], F32, tag="ws",
                                 name=f"ws_wq_{h}_{j}")
                eng = nc.scalar if h < 2 else nc.sync
                eng.dma_start(out=ws, in_=wq_src[:, h, j])
                wq_st.append((h, j, ws))
        wk_st = w_dma(awk, DB // 256, H * DK, "wk", nc.scalar)
        # wv: plain kt-major halves for the bf16 v-projection
        wv_src = awv.rearrange("(t u) c -> u t c", t=DB // P, u=P)
        wv_st = []
        for i in range(DB // P):
            ws = wstage.tile([P, H * DV], F32, tag="ws", name=f"ws_wv_{i}")
            nc.scalar.dma_start(out=ws, in_=wv_src[:, i, :])
            wv_st.append(ws)

        # ---- q marshal: two 8KB pieces through the kst staging ring ------
        qsrc = aq.rearrange("(s p) d -> p s d", p=P)
        qpieces = []
        for i in range(2):
            qp = cstage.tile([P, 2, DL], F32, tag="vst", name=f"qst{i}")
            nc.gpsimd.dma_start(out=qp, in_=qsrc[:, 2 * i:2 * i + 2, :])
            qpieces.append(qp)

        # ---- k/v chunk marshal pieces ------------------------------------
        def marshal_dma(c):
            kst = cstage.tile([P, MC // P, DB], F32, tag="cst", name=f"kst{c}",
                              bufs=3)
            nc.gpsimd.dma_start(
                out=kst, in_=ak[c * MC:(c + 1) * MC, :].rearrange(
                    "(s p) d -> p s d", p=P))
            vst = cstage.tile([P, MC // P, DB], F32, tag="vst", name=f"vst{c}")
            nc.sync.dma_start(
                out=vst, in_=av[c * MC:(c + 1) * MC, :].rearrange(
                    "(s p) d -> p s d", p=P))
            return kst, vst

        def marshal_rest(c, kst, vst):
            # staging is bf16-typed (fp8 pair units); cast regroups halves:
            # x8[p, half, s*256+x] = x[s*128+p, half*256+x]
            k8c = c8p.tile([P, DB // 256, (MC // P) * P], BF16, tag="c8",
                           name=f"k8{c}")
            v16c = c8p.tile([P, MC // P, DB], BF16, tag="v16", name=f"v16{c}")
            nc.vector.tensor_copy(out=v16c, in_=vst)
            for half in range(DB // 256):
                nc.vector.tensor_copy(
                    out=k8c[:, half].bitcast(FP8).rearrange(
                        "p (s x) -> p s x", s=MC // P),
                    in_=kst[:, :, half * 256:(half + 1) * 256])

            # vT_bf[db%128, 4*s + db//128, m127] = v[c*512 + s*128 + m127, db]
            vT8c = vT8p.tile([P, (MC // P) * (DB // P), P], BF16, tag="vT",
                             name=f"vT{c}")
            for half in range(DB // 256):
                nc.sync.dma_start_transpose(out=kT8u[:, half, c],
                                            in_=k8c[:, half])
            nc.sync.dma_start_transpose(out=vT8c, in_=v16c)
            return vT8c

        # DVE stream head: q casts (data ~10us), then weight casts in
        # arrival order.
        # q8[p, half, s*256+x] = q[s*128+p, half*256+x]  (fp8 in bf16 units)
        for half in range(DL // 256):
            for i, qp in enumerate(qpieces):
                nc.vector.tensor_copy(
                    out=q8[:, half].bitcast(FP8).rearrange(
                        "p (s x) -> p s x", s=N // P)[:, 2 * i:2 * i + 2, :],
                    in_=qp[:, :, half * 256:(half + 1) * 256])
            nc.sync.dma_start_transpose(out=qT8u[:, half],
                                        in_=q8[:, half])
        for h, j, ws in wq_st:
            nc.vector.tensor_scalar_mul(wq8[:, h, j], ws, WS)
        for h, j, ws in wk_st:
            nc.vector.tensor_scalar_mul(wk8[:, h, j], ws, WS)
        for i, ws in enumerate(wv_st):
            nc.scalar.activation(out=wv16[:, i, :], in_=ws, func=COPY,
                                 scale=WS)

        # ---- Q projection (DR): qhT[h] = (q @ 8 w_q)^T -------------------
        for h in range(H):
            psQ = misc.tile([P, N], F32, tag="misc", name=f"psQ{h}")
            for half in range(DL // 256):
                nc.tensor.matmul(
                    psQ,
                    lhsT=wq8[:, half, :, h * DK:(h + 1) * DK],
                    rhs=_dr_rhs(qT8u[:, half]),
                    start=(half == 0), stop=(half == DL // 256 - 1),
                    perf_mode=DR,
                )
            nc.scalar.activation(out=qhT[:, h, :], in_=psQ, func=COPY)

        # ---- kh projection for (head, chunk): 2 DR MMs + bf16 evac -------
        def kh_chunk(kht_dst, h, c):
            psK = misc.tile([P, MC], F32, tag="misc", name=f"psK{h}_{c}")
            for half in range(DB // 256):
                nc.tensor.matmul(
                    psK,
                    lhsT=wk8[:, half, :, h * DK:(h + 1) * DK],
                    rhs=_dr_rhs(kT8u[:, half, c]),
                    start=(half == 0), stop=(half == DB // 256 - 1),
                    perf_mode=DR,
                )
            nc.vector.tensor_copy(out=kht_dst[:, c * MC:(c + 1) * MC], in_=psK)

        # ---- v projection for one chunk: vh[ms in c, :] ------------------
        def v_chunk(vT8c, c):
            for msl in range(MC // P):
                ms = c * (MC // P) + msl
                for oc in range(H * DV // 512):
                    psV = misc.tile([P, 512], F32, tag="misc",
                                    name=f"psV{ms}_{oc}")
                    for kt in range(DB // P):
                        nc.tensor.matmul(
                            psV,
                            lhsT=vT8c[:, 4 * msl + kt, :],
                            rhs=wv16[:, kt, oc * 512:(oc + 1) * 512],
                            start=(kt == 0), stop=(kt == DB // P - 1),
                        )
                    nc.vector.tensor_copy(
                        out=vh[:, ms, oc * 512:(oc + 1) * 512], in_=psV)

        # ---- attention ---------------------------------------------------
        kht_cur = khtp.tile([P, M], FP8, tag="kht", name="kht0")
        pending = []

        for h in range(H):
            if h == 1:
                # w_o: DMA on the (now idle-ish) sync queue, cast on gpsimd
                # (after all marshal casts) -- ready long before out-proj.
                wo_st = w_dma(awo, (H * DV) // 256, DL, "wo", nc.scalar,
                              pat="(h j u) c -> u h j c")
                for hh, j, ws in wo_st:
                    nc.gpsimd.tensor_copy(out=wo8[:, hh, j], in_=ws)

            psU = psUp.tile([P, N], F32, tag="psU", name=f"psU{h}")
            psr = psRp.tile([1, N], F32, tag="psr", name=f"psr{h}")
            ets = [None] * NG
            kht_nxt = (khtp.tile([P, M], FP8, tag="kht", name=f"kht{h + 1}")
                       if h + 1 < H else None)

            def pv_rowsum(g, psU=psU, psr=psr, ets=ets, h=h):
                nc.tensor.matmul(
                    psU,
                    lhsT=vh[:, 2 * g:2 * g + 2, h * DV:(h + 1) * DV],
                    rhs=ets[g],
                    start=(g == 0), stop=(g == NG - 1),
                    perf_mode=DR,
                )
                nc.tensor.matmul(
                    psr,
                    lhsT=ones8[:, :, 0:1],
                    rhs=ets[g],
                    start=(g == 0), stop=(g == NG - 1),
                    perf_mode=DR,
                )

            for g in range(NG):
                c = g // 2
                if h == 0 and g % 2 == 0:
                    if g == 0:
                        st = [marshal_dma(0), marshal_dma(1)]
                        vts = [marshal_rest(0, *st[0])]
                    if c + 2 < NCH:
                        st.append(marshal_dma(c + 2))
                    if c + 1 < NCH and len(vts) == c + 1:
                        vts.append(marshal_rest(c + 1, *st[c + 1]))
                    kh_chunk(kht_cur, 0, c)
                    v_chunk(vts[c], c)
                    kh_chunk(kht_nxt, 1, c)
                elif h > 0 and kht_nxt is not None and g % 2 == 0:
                    kh_chunk(kht_nxt, h + 1, c)

                psS = psSp.tile([P, 2, N], F32, tag="psS")
                for j in range(2):
                    mt = 2 * g + j
                    nc.tensor.matmul(
                        psS[:, j, :],
                        lhsT=kht_cur[:, mt * P:(mt + 1) * P],
                        rhs=qhT[:, h, :],
                        start=True, stop=True,
                    )
                et = etp.tile([P, 2, N], FP8, tag="et")
                nc.scalar.activation(out=et, in_=psS, func=EXP,
                                     scale=TEMP / (WS * WS), bias=biasT)
                ets[g] = et
                if pending:
                    pending.pop(0)()
                if g >= LAGG:
                    pv_rowsum(g - LAGG)

            def normalize(psU=psU, psr=psr, h=h):
                rec = recp.tile([1, N], F32, tag="rec", name=f"rec{h}")
                nc.vector.reciprocal_approx_fast(out=rec, in_=psr)
                recb = recp.tile([1, N], BF16, tag="recb", name=f"recb{h}")
                nc.vector.tensor_copy(out=recb, in_=rec)
                psRec = misc.tile([P, N], F32, tag="misc", name=f"psRec{h}")
                nc.tensor.matmul(psRec, lhsT=onesr, rhs=recb,
                                 start=True, stop=True)
                usb = recp.tile([P, N], BF16, tag="usb", name=f"usb{h}")
                nc.vector.tensor_copy(out=usb, in_=psU)
                nc.vector.tensor_tensor(out=UT8[:, h, :], in0=psRec, in1=usb,
                                        op=MULT)

            pending = [
                (lambda g=g, f=pv_rowsum: f(g)) for g in range(NG - LAGG, NG)
            ] + [normalize]
            if h == H - 1:
                for fn in pending:
                    fn()
            kht_cur = kht_nxt

        # ---- output projection + residual (DR over head pairs) -----------
        for nt in range(N // P):
            for oc in range(DL // 512):
                psO = misc.tile([P, 512], F32, tag="misc", name=f"psO{nt}_{oc}")
                for hh in range((H * DV) // 256):
                    nc.tensor.matmul(
                        psO,
                        lhsT=UT8[:, 2 * hh:2 * hh + 2, nt * P:(nt + 1) * P],
                        rhs=wo8[:, hh, :, oc * 512:(oc + 1) * 512],
                        start=(hh == 0), stop=(hh == (H * DV) // 256 - 1),
                        perf_mode=DR,
                    )
                qres = otp.tile([P, 512], F32, tag="qres")
                nc.gpsimd.dma_start(
                    out=qres,
                    in_=aq[nt * P:(nt + 1) * P, oc * 512:(oc + 1) * 512])
                ot = otp.tile([P, 512], F32, tag="ot")
                nc.vector.scalar_tensor_tensor(
                    out=ot, in0=psO, scalar=1.0 / (WS ** 2),
                    in1=qres, op0=MULT, op1=ADD)
                nc.gpsimd.dma_start(
                    out=aout[nt * P:(nt + 1) * P, oc * 512:(oc + 1) * 512],
                    in_=ot)

        if DEBUG_DUMP is not None:
            DEBUG_DUMP(nc, locals())


_CACHE = {}


def _get_nc():
    if "nc" not in _CACHE:
        nc = bacc.Bacc("TRN2", target_bir_lowering=False, debug=False)
        with tile.TileContext(nc) as tc:
            build_kernel(nc, tc)
        nc.compile()
        _CACHE["nc"] = nc
    return _CACHE["nc"]


def kernel(q, k, v, w_q, w_k, w_v, w_o):
    from concourse.bass_utils import run_bass_kernel_spmd

    nc = _get_nc()
    in_maps = []
    for i in range(B):
        in_maps.append({
            "q": np.ascontiguousarray(q[i], dtype=np.float32),
            "k": np.ascontiguousarray(k[i], dtype=np.float32),
            "v": np.ascontiguousarray(v[i], dtype=np.float32),
            "w_q": np.ascontiguousarray(w_q, dtype=np.float32),
            "w_k": np.ascontiguousarray(w_k, dtype=np.float32),
            "w_v": np.ascontiguousarray(w_v, dtype=np.float32),
            "w_o": np.ascontiguousarray(w_o, dtype=np.float32),
        })
    res = run_bass_kernel_spmd(nc, in_maps, core_ids=list(range(B)))
    return np.stack([res.results[i]["out"] for i in range(B)], axis=0)
